# revision 86
# baseline (speedup 1.0000x reference)
"""Fused multi-head attention forward for TRN2, SPMD over 8 NeuronCores.

Problem: B=2, S=2048, D=1024, H=16 heads (Hd=64), fp32.
  out = proj(softmax((x@Wq + bq)(x@Wk + bk)^T / 8) @ (x@Wv + bv))

Sharding: 2-way data parallel over batch x 4-way tensor parallel over heads.
Core c handles batch c//4 and heads [4*(c%4), 4*(c%4)+4). Attention is fully
local; the output projection is computed on each core over its 256 head
features (with bias/4) into a full [S, D] partial; the host sums the four
partials per batch during unshard.

v8 vs v2 (187.3 us -> 160.6 us TimelineSim, device rel err 5.4e-3):
- The v2 trace shows ACT (exp) busy 138 us but idle for the first 37 us
  because all of phase A (QKV+V projections) was emitted before the first
  score matmul. v4 removes phase A: a minimal prefix (Q pair0 chunk0 +
  K pair0 chunk0) starts the scores->exp pipeline as soon as the x0/wqk
  DMAs land (~12 us, DMA-bandwidth-bound), and every remaining qk chunk /
  V half-tile / projection j-half is placed by a deadline-driven
  latest-fit placer into per-group filler slots (chained units keep a
  chunk's half-0 ahead of its half-1).
- V tiles split into per-pair halves so pair-1 halves inherit deadlines
  16 groups later, easing the early PE crunch.
- DELAY (scores->av lag) raised 6 -> 38 with a 40-slot exp ring
  (80KB/partition SBUF): the deep ring decouples the av stream from exp
  jitter, which removed ~13 us of accumulated pipeline stalls. PE and
  ACT are balanced in-loop (PE ~131 us vs ACT ~133 us of exp), so any
  residual stall surfaces 1:1 in the span; larger DELAY absorbs them.
- The tail (last chunk after the final exp) avoids the DMA-transpose
  path entirely: otq blocks are transposed on the PE against an identity
  tile into bf16 PSUM and drained by the otherwise-idle DVE, the
  projection folds the bias on the PE, and one ACT copy per row block
  drains to SBUF with stores on the idle SP queue. The serial-HWDGE +
  0.9us DMA-semaphore cost of a transpose would sit fully exposed here.
- Output stores issue on the Pool queue (SWDGE) mid-pipeline so the
  norm DMA-transposes never queue behind them on SP.SEQ.
- av emission catches up ~28 extra avs over the filler-free last ~47
  groups (CATCH/CSTART), pulling norm(1,2) and the qc2 projections back
  inside the loop; only ~10 avs + the last chunk's norm remain in the
  flush after the final exp.
- In the tail, the PE-transpose staging tiles live in the (post-loop
  idle) pp pool, not sp: with sp holding only the projection psums, the
  transposes can run one block ahead and proj(s) finds its ot drain
  already complete (sp rotation then pairs proj psums with ACT copies
  two blocks apart, which never blocks).
- A junk-matmul warm-up ramps the PE p-state (0.65 -> 2.4 GHz needs a
  3us busy streak) while the first input DMAs are still in flight.
- The host packs wqk columns as [Q-pair0 | K-pair0 | Q-pair1 | K-pair1]
  (WCOL remap in qk_half) so the prefix's weight DMA is one contiguous
  256-column piece: 3MB instead of 4MB lands before the first exp, while
  descriptors stay at the efficient 1KB size (column-sliced pieces of
  the old layout transferred ~35% slower).
- x stripes 1-2 are promoted ahead of the Q1/K1 weight block in the DMA
  queue and x3 ahead of wv: transfers are effectively serial, and at
  DELAY=38 the weights/wv have far more deadline slack than the K-pair0
  chunks that stall the early exps waiting for x (-1.5us).

Engine budget per core (TimelineSim): exp stream 12.9 + 136.4 us
(ACT-bound, ~2 us stalls), tail 13.3 us. PE busy ~143, ACT 138.3.
Remaining levers are structural: exp floor is 0.833ns/elem on ACT
(131072 elems/partition), PE floor ~138 us in bf16 (fp8 DoubleRow needs
both operands fp8; softmax/attention precision and the 16KB PSUM wall
rule out every packing variant that was analyzed).
"""
import os
import sys

sys.path.insert(0, "/opt/trn_rl_repo")
from collections import defaultdict
from contextlib import ExitStack

import numpy as np

import concourse.bass as bass
import concourse.tile as tile
from concourse import bacc, mybir
from concourse.bass_utils import run_bass_kernel_spmd
from concourse.masks import make_identity

F32 = mybir.dt.float32
BF16 = mybir.dt.bfloat16
EXP = mybir.ActivationFunctionType.Exp
COPY = mybir.ActivationFunctionType.Copy

P = 128
B, S, D, H, HD = 2, 2048, 1024, 16, 64
NH = 4          # heads per core
FQ = NH * HD    # 256 q/k/v features per core
ST = S // P     # 16 seq tiles
KD = D // P     # 8 contraction tiles over d_model
QC = 4          # q chunks
QW = S // QC    # 512
N_CORES = 8
DELAY = int(os.environ.get("MHA_DELAY", "38"))  # emit_scores(g) -> emit_av(g)
RING = DELAY + 2                                # exp ring slots
B_EARLY = float(os.environ.get("MHA_BE", "0.50"))
B_STEADY = float(os.environ.get("MHA_BS", "0.40"))
VCAP = int(os.environ.get("MHA_VCAP", "99"))


def build():
    nc = bacc.Bacc(
        "TRN2",
        target_bir_lowering=False,
        debug=False,
        enable_asserts=False,
        num_devices=N_CORES,
    )
    xt_d = nc.dram_tensor("xt", [D, S], F32, kind="ExternalInput").ap()
    wqk_d = nc.dram_tensor("wqk", [D, 2 * FQ], F32, kind="ExternalInput").ap()
    wv_d = nc.dram_tensor("wv", [D, FQ], F32, kind="ExternalInput").ap()
    bqk_d = nc.dram_tensor("bqk", [2 * FQ, 1], F32, kind="ExternalInput").ap()
    bv_d = nc.dram_tensor("bv", [1, FQ], F32, kind="ExternalInput").ap()
    wpr_d = nc.dram_tensor("wpr", [FQ, D], F32, kind="ExternalInput").ap()
    bpr_d = nc.dram_tensor("bpr", [1, D], F32, kind="ExternalInput").ap()
    out_d = nc.dram_tensor("out", [S, D], BF16, kind="ExternalOutput").ap()

    with tile.TileContext(nc) as tc, ExitStack() as ctx:
        const = ctx.enter_context(tc.tile_pool(name="const", bufs=1))
        qkv = ctx.enter_context(tc.tile_pool(name="qkv", bufs=1))
        otp = ctx.enter_context(tc.tile_pool(name="otp", bufs=1))
        mis = ctx.enter_context(tc.tile_pool(name="mis", bufs=2))
        otqp = ctx.enter_context(tc.tile_pool(name="otqp", bufs=10))
        pp = ctx.enter_context(tc.tile_pool(name="pp", bufs=2, space="PSUM"))
        xa = ctx.enter_context(tc.tile_pool(name="xt", bufs=1))
        wa = ctx.enter_context(tc.tile_pool(name="wa", bufs=1))

        # ---- small bias DMAs first (SP/HWDGE, cheap, parallel to the Pool
        # SWDGE generation stream), needed at first drains
        bv_s = const.tile([1, FQ], F32)
        nc.sync.dma_start(bv_s[:], bv_d[:])
        bqk_s = []
        for m in range(4):
            t = const.tile([P, 1], F32, name=f"bqk{m}")
            nc.sync.dma_start(t[:], bqk_d[m * P : (m + 1) * P, :])
            bqk_s.append(t)

        # ---- inputs via batched SWDGE casting DMAs (f32 DRAM -> bf16 SBUF),
        # ordered so the prefix (Q pair0 chunk0, K pair0 chunk0) unblocks
        # first: Q-columns + x stripe 0 lead, K-columns next, then wv and the
        # remaining x stripes.
        xt3 = [xa.tile([P, KD, QW], BF16, name=f"x{c}") for c in range(QC)]
        wqk3 = wa.tile([P, KD, 2 * FQ], BF16, name="wqk")
        wv3 = wa.tile([P, KD, FQ], BF16, name="wv")
        xt_v = xt_d.rearrange("(k p) s -> p k s", p=P)
        wqk_v = wqk_d.rearrange("(k p) f -> p k f", p=P)

        # host packs wqk columns [Q0 | K0 | Q1 | K1]: the prefix piece is
        # the contiguous first 256 columns, interleaved with x0 halves so
        # the first score group unblocks as early as the DMA engines allow
        KH = KD // 2
        nc.gpsimd.dma_start(wqk3[:, 0:KH, 0:FQ], wqk_v[:, 0:KH, 0:FQ])
        nc.gpsimd.dma_start(xt3[0][:, 0:KH, :], xt_v[:, 0:KH, 0:QW])
        nc.gpsimd.dma_start(wqk3[:, KH:, 0:FQ], wqk_v[:, KH:, 0:FQ])
        nc.gpsimd.dma_start(xt3[0][:, KH:, :], xt_v[:, KH:, 0:QW])
        nc.gpsimd.dma_start(xt3[1][:], xt_v[:, :, QW : 2 * QW])
        nc.gpsimd.dma_start(xt3[2][:], xt_v[:, :, 2 * QW : 3 * QW])
        nc.gpsimd.dma_start(wqk3[:, :, FQ:], wqk_v[:, :, FQ:])
        nc.gpsimd.dma_start(xt3[3][:], xt_v[:, :, 3 * QW : 4 * QW])
        nc.gpsimd.dma_start(wv3[:], wv_d.rearrange("(k p) f -> p k f", p=P))
        wpr3 = wa.tile([P, 2, D], BF16, name="wpr")
        nc.gpsimd.dma_start(wpr3[:], wpr_d.rearrange("(j p) f -> p j f", p=P))
        bpr_s = const.tile([1, D], BF16)
        nc.gpsimd.dma_start(bpr_s[:], bpr_d[:])

        wqk_s = [wqk3[:, k, :] for k in range(KD)]
        wpr_s = [wpr3[:, j, :] for j in range(2)]

        # ---- small constants
        ones_f = const.tile([1, P], F32)
        nc.vector.memset(ones_f[:], 1.0)
        ones128 = const.tile([1, P], BF16)
        nc.vector.tensor_copy(ones128[:], ones_f[:])
        ident = const.tile([P, P], BF16, name="ident")
        make_identity(nc, ident[:])
        onesv = const.tile([P, ST, NH, 1], BF16)
        nc.vector.memset(onesv[:], 1.0)
        bias_bcast = const.tile([P, D], F32)

        qt_t = [qkv.tile([P, S], BF16, name=f"qt{i}") for i in range(2)]
        kt_t = [qkv.tile([P, S], BF16, name=f"kt{i}") for i in range(2)]
        vt_t = qkv.tile([P, ST, NH, HD + 1], BF16, name="vt")
        nc.vector.tensor_copy(vt_t[:, :, :, HD : HD + 1], onesv[:])

        # bv broadcast target (filled right after the prefix; each v_half
        # folds the bias into its PSUM-drain DVE op)
        bv_bcast = const.tile([P, FQ], F32, name="bvb")

        qk_pending = {}
        # wqk column block per m-tile (host packs [Q0 | K0 | Q1 | K1])
        WCOL = {0: 0, 1: 2, 2: 1, 3: 3}

        def qk_half(m, qc, half):
            # m-tile -> destination: 0,1 = Q pairs; 2,3 = K pairs
            if half == 0:
                qk_pending[(m, qc)] = pp.tile([P, QW], F32, name="pp")
            pq = qk_pending[(m, qc)]
            for k in range(half * KD // 2, (half + 1) * KD // 2):
                nc.tensor.matmul(
                    pq[:],
                    wqk_s[k][:, WCOL[m] * P : (WCOL[m] + 1) * P],
                    xt3[qc][:, k, :],
                    start=(k == 0),
                    stop=(k == KD - 1),
                )
            if half == 1:
                dest = qt_t[m] if m < 2 else kt_t[m - 2]
                nc.vector.tensor_scalar_add(
                    dest[:, qc * QW : (qc + 1) * QW], pq[:], bqk_s[m][:]
                )
                del qk_pending[(m, qc)]

        def v_half(st, pr):
            # V projection for sequence tile st, head pair pr (128 features)
            pv = pp.tile([P, FQ // 2], F32, name="pp")
            c, r = divmod(st * P, QW)
            cols = slice(pr * P, (pr + 1) * P)
            for k in range(KD):
                nc.tensor.matmul(
                    pv[:],
                    xt3[c][:, k, r : r + P],
                    wv3[:, k, cols],
                    start=(k == 0),
                    stop=(k == KD - 1),
                )
            nc.vector.tensor_add(
                vt_t[:, st, 2 * pr : 2 * pr + 2, 0:HD],
                pv[:].rearrange("p (a b) -> p a b", a=2),
                bv_bcast[:, cols].rearrange("p (a b) -> p a b", a=2),
            )

        # ---- attention pipeline state
        ot_t = [otp.tile([P, S], BF16, name=f"ot{i}") for i in range(2)]

        proj_out = {}

        def proj_half(qc, sub, j):
            # one j-half of a projection sub-block: 2 matmuls + DVE bias-add
            qt = qc * 4 + sub
            ts = slice(qt * P, (qt + 1) * P)
            if j == 0:
                proj_out[(qc, sub)] = mis.tile([P, D], BF16, name="outsb")
            outsb = proj_out[(qc, sub)]
            js = slice(j * QW, (j + 1) * QW)
            ppp = pp.tile([P, QW], F32, name="pp")
            nc.tensor.matmul(
                ppp[:], ot_t[0][:, ts], wpr_s[0][:, js], start=True, stop=False
            )
            nc.tensor.matmul(
                ppp[:], ot_t[1][:, ts], wpr_s[1][:, js], start=False, stop=True
            )
            nc.vector.tensor_add(outsb[:, js], ppp[:], bias_bcast[:, js])
            if j == 1:
                nc.gpsimd.dma_start(out_d[ts, :], outsb[:])
                del proj_out[(qc, sub)]

        def proj_sub(qc, sub, tail=False, split_store=False):
            qt = qc * 4 + sub
            ts = slice(qt * P, (qt + 1) * P)
            outsb = mis.tile([P, D], BF16, name="outsb")
            if tail:
                # post-last-exp: DVE is busy with the norm muls/drains, ACT
                # is idle -> fold bias on the PE, drain with one ACT copy.
                # The last blocks split drain+store per j-half so the final
                # copy starts as soon as its j's accumulation stops and the
                # last DMA transfer is half-sized.
                pt = sp.tile([P, 2 * QW], F32, name="ps")
                for j in range(2):
                    js = slice(j * QW, (j + 1) * QW)
                    nc.tensor.matmul(
                        pt[:, js], ot_t[0][:, ts], wpr_s[0][:, js],
                        start=True, stop=False,
                    )
                    nc.tensor.matmul(
                        pt[:, js], ot_t[1][:, ts], wpr_s[1][:, js],
                        start=False, stop=False,
                    )
                    nc.tensor.matmul(
                        pt[:, js], ones128[:], bpr_s[0:1, js],
                        start=False, stop=True,
                    )
                    if split_store:
                        nc.scalar.activation(
                            outsb[:, js], pt[:, js], COPY, bias=0.0, scale=1.0
                        )
                        nc.sync.dma_start(out_d[ts, js], outsb[:, js])
                if not split_store:
                    nc.scalar.activation(
                        outsb[:], pt[:], COPY, bias=0.0, scale=1.0
                    )
                    nc.sync.dma_start(out_d[ts, :], outsb[:])
                return
            for j in range(2):
                js = slice(j * QW, (j + 1) * QW)
                ppp = pp.tile([P, QW], F32, name="pp")
                nc.tensor.matmul(
                    ppp[:], ot_t[0][:, ts], wpr_s[0][:, js],
                    start=True, stop=False,
                )
                nc.tensor.matmul(
                    ppp[:], ot_t[1][:, ts], wpr_s[1][:, js],
                    start=False, stop=True,
                )
                nc.vector.tensor_add(outsb[:, js], ppp[:], bias_bcast[:, js])
            nc.gpsimd.dma_start(out_d[ts, :], outsb[:])

        def bias_fn():
            # bias_bcast[p, n] = b_proj[n] (pre-scaled by 1/4 on host)
            for j in range(2):
                pb = pp.tile([P, QW], F32, name="pp")
                nc.tensor.matmul(
                    pb[:], ones128[:], bpr_s[0:1, j * QW : (j + 1) * QW],
                    start=True, stop=True,
                )
                nc.vector.tensor_copy(bias_bcast[:, j * QW : (j + 1) * QW], pb[:])

        with ExitStack() as ctx_b:
            att = ctx_b.enter_context(tc.tile_pool(name="att", bufs=1))
            sp = ctx_b.enter_context(tc.tile_pool(name="sp", bufs=2, space="PSUM"))
            op = ctx_b.enter_context(tc.tile_pool(name="op", bufs=1, space="PSUM"))

            at = att.tile([P, RING, 2 * QW], BF16, name="at")
            po_cur = {}

            def emit_scores(g, qc, p, kt):
                qs = slice(qc * QW, (qc + 1) * QW)
                ks = slice(kt * P, (kt + 1) * P)
                ps = sp.tile([P, 2 * QW], F32, name="ps")
                nc.tensor.matmul(
                    ps[:, 0:QW], kt_t[p][0:64, ks], qt_t[p][0:64, qs],
                    start=True, stop=True, tile_position=(0, 0),
                )
                nc.tensor.matmul(
                    ps[:, QW : 2 * QW], kt_t[p][64:128, ks], qt_t[p][64:128, qs],
                    start=True, stop=True, tile_position=(64, 0),
                )
                nc.scalar.activation(
                    at[:, g % RING, :], ps[:], EXP, bias=0.0, scale=0.125
                )

            def norm_sub(p, qc, po0, po1, recips, s, pe_transpose=False):
                ts = slice(qc * QW + s * P, qc * QW + (s + 1) * P)
                otq = otqp.tile([P, 2, HD], BF16, name="otq")
                nc.vector.tensor_scalar_mul(
                    otq[:, 0, :], po0[:, s, 0:HD], recips[:, 0, s, :]
                )
                nc.vector.tensor_scalar_mul(
                    otq[:, 1, :], po1[:, s, 0:HD], recips[:, 1, s, :]
                )
                if pe_transpose:
                    # tail path: the DMA-transpose (serial HWDGE gen + 0.9us
                    # DMA-completion semaphore) is too slow on the critical
                    # tail; transpose on the PE and drain via idle DVE
                    # pp pool is idle post-loop; keeping the T staging out of
                    # sp lets the proj psum tiles rotate against ACT copies
                    # only, so transposes can run ahead of the projections
                    tp = pp.tile([P, P], BF16, name="pp")
                    nc.tensor.transpose(tp[0:64, :], otq[:, 0, :], ident[:])
                    nc.tensor.transpose(tp[64:128, :], otq[:, 1, :], ident[:])
                    nc.vector.tensor_copy(ot_t[p][:, ts], tp[:])
                else:
                    nc.sync.dma_start(ot_t[p][:, ts], otq[:], transpose=True)

            def emit_norm(p, qc, tail=False):
                po0, po1 = po_cur.pop((p, qc))
                recips = otqp.tile([P, 2, NH, 1], F32, name="recips")
                with nc.allow_low_precision(reason="softmax recip"):
                    nc.vector.reciprocal(recips[:, 0, :, :], po0[:, :, HD : HD + 1])
                    nc.vector.reciprocal(recips[:, 1, :, :], po1[:, :, HD : HD + 1])
                if tail:
                    # transposes one block ahead: proj(s) finds its drain done
                    norm_sub(p, qc, po0, po1, recips, 0, pe_transpose=True)
                    for s in range(1, 4):
                        norm_sub(p, qc, po0, po1, recips, s, pe_transpose=True)
                        proj_sub(qc, s - 1, tail=True)
                    proj_sub(qc, 3, tail=True)
                    return
                # hh-major mul order: po0's region is fully read after 4 muls
                # (not 7), so the next chunk's first av unblocks ~0.6us sooner
                otqs = [otqp.tile([P, 2, HD], BF16, name="otq") for _ in range(4)]
                for s in range(4):
                    nc.vector.tensor_scalar_mul(
                        otqs[s][:, 0, :], po0[:, s, 0:HD], recips[:, 0, s, :]
                    )
                for s in range(4):
                    nc.vector.tensor_scalar_mul(
                        otqs[s][:, 1, :], po1[:, s, 0:HD], recips[:, 1, s, :]
                    )
                for s in range(4):
                    ts = slice(qc * QW + s * P, qc * QW + (s + 1) * P)
                    nc.sync.dma_start(ot_t[p][:, ts], otqs[s][:], transpose=True)

            def emit_av(g, qc, p, kt):
                if kt == 0:
                    po_cur[(p, qc)] = (
                        op.tile([P, NH, P], F32, name="po0"),
                        op.tile([P, NH, P], F32, name="po1"),
                    )
                po0, po1 = po_cur[(p, qc)]
                for hh, po in ((0, po0), (1, po1)):
                    for s in range(4):
                        # sub-regions padded to the 512B PSUM zero-region
                        # so each accumulation group owns its region cleanly
                        nc.tensor.matmul(
                            po[:, s, 0 : HD + 1],
                            at[:, g % RING, hh * QW + s * P : hh * QW + (s + 1) * P],
                            vt_t[:, kt, 2 * p + hh, :],
                            start=(kt == 0 and s == 0),
                            stop=(kt == ST - 1),
                            skip_group_check=True,
                        )
                if kt == ST - 1:
                    emit_norm(p, qc, tail=(p == 1 and qc == QC - 1))

            seq = [
                (qc, p, kt)
                for qc in range(QC)
                for p in range(2)
                for kt in range(ST)
            ]

            # ---- deadline-driven filler schedule -------------------------
            # Each unit: (latest_emit_group, est_PE_us, chain, fn). Latest-fit
            # into per-group budgets; overflow spills into the prefix. Units
            # sharing a chain are re-bound to their assigned slots in order,
            # so a chunk's half-1 never executes before its half-0.
            units = []

            def qkh(m, qc, h):
                return lambda: qk_half(m, qc, h)

            def vh(st, pr):
                return lambda: v_half(st, pr)

            def prj(qc, s_, j_):
                return lambda: proj_half(qc, s_, j_)

            def add_qk(m, qc, e):
                # -2 margin: the chunk's DVE bias-add drain + semaphore chain
                # land ~1 group after the PE half finishes
                units.append([e - 3, 0.85, (m, qc), qkh(m, qc, 0)])
                units.append([e - 2, 0.85, (m, qc), qkh(m, qc, 1)])

            # K pair0 chunks 1-3 (chunk c first used by scores group 4c)
            for c in (1, 2, 3):
                add_qk(2, c, 4 * c - 1)
            # Q pair1 qc0 + K pair1 chunks (first used at group 16 + 4c)
            add_qk(1, 0, 15)
            for c in range(4):
                add_qk(3, c, 16 + 4 * c - 1)
            # Q chunks 1-3 for both pairs (first used at 32qc / 32qc+16)
            for qc in (1, 2, 3):
                add_qk(0, qc, 32 * qc - 1)
                add_qk(1, qc, 32 * qc + 15)
            # V halves: pair0 feeds av(qc0,p0,st) at group st+DELAY; pair1
            # feeds av(qc0,p1,st) at group 16+st+DELAY (capped before projs)
            for st in range(ST):
                units.append([st + DELAY - 2, 0.43, None, vh(st, 0)])
                units.append(
                    [min(st + 16 + DELAY - 2, VCAP), 0.43, None, vh(st, 1)]
                )
            units.append([34, 0.45, None, bias_fn])

            NG = len(seq)
            budget = [B_EARLY if g < 28 else B_STEADY for g in range(NG)]
            sched = defaultdict(list)
            # fixed-position projection j-halves: norm(1,qc) is emitted at
            # loop group 32qc+31+DELAY; spread the 8 halves right after, and
            # pre-charge their PE cost so the placer avoids those groups
            # av emission trails scores by DELAY; optionally catch up CATCH
            # extra avs over the filler-free last groups so norm(1,2) and the
            # qc2 projections land inside the loop instead of the flush
            CATCH = int(os.environ.get("MHA_CATCH", "28"))
            CSTART = int(os.environ.get("MHA_CSTART", "80"))

            def av_due(g):
                d = g - DELAY
                if CATCH and g > CSTART:
                    d += min(CATCH, ((g - CSTART) * CATCH) // (NG - 1 - CSTART))
                return min(d, g)

            def norm_emit_group(av_idx):
                for g_ in range(NG):
                    if av_due(g_) >= av_idx:
                        return g_
                return NG + (av_idx - av_due(NG - 1))

            post_loop = defaultdict(list)
            for qc in range(3):
                base = norm_emit_group(32 * qc + 31) + 1
                for s_ in range(4):
                    for j_ in range(2):
                        g_ = base + 2 * s_ + j_
                        if g_ < NG:
                            sched[g_].append(prj(qc, s_, j_))
                            budget[g_] -= 0.53
                        else:
                            post_loop[g_ - NG].append(prj(qc, s_, j_))
            placed = []  # (group or -1 for prefix, order, chain, fn)
            for e, cost, chain, fn in sorted(units, key=lambda u: u[0]):
                g = min(e, NG - 1)
                while g >= 0 and budget[g] <= 1e-9:
                    g -= 1
                if g >= 0:
                    budget[g] -= cost
                placed.append([g, chain, fn])
            # re-bind chained units: sort each chain's slots, keep fn order
            by_chain = defaultdict(list)
            for i, (g, chain, fn) in enumerate(placed):
                if chain is not None:
                    by_chain[chain].append(i)
            for idxs in by_chain.values():
                slots = sorted(placed[i][0] for i in idxs)
                for i, s_ in zip(idxs, slots):
                    placed[i][0] = s_
            prefix_units = []
            for g, chain, fn in placed:
                if g < 0:
                    prefix_units.append(fn)
                else:
                    sched[g].append(fn)

            # ---- PE warm-up: junk matmuls on constants while the first x/w
            # DMAs are in flight, so the p-state ramp (0.65 -> 2.4 GHz over a
            # 3us busy streak) completes before the real prefix work starts
            junk = sp.tile([P, 2 * QW], F32, name="ps")
            for _ in range(int(os.environ.get("MHA_WARM", "26"))):
                nc.tensor.matmul(
                    junk[:, 0:P], ones128[:], ones128[:], start=True, stop=True
                )

            # ---- prefix: just enough to start the pipeline; both halves
            # of each k-piece carry Q0 and K0 columns, so interleave
            qk_half(0, 0, 0)   # Q pair0 chunk0, k 0-3
            qk_half(2, 0, 0)   # K pair0 chunk0, k 0-3
            qk_half(0, 0, 1)
            qk_half(2, 0, 1)
            pb0 = pp.tile([P, FQ], F32, name="pp")
            nc.tensor.matmul(pb0[:], ones_f[:], bv_s[:], start=True, stop=True)
            nc.vector.tensor_copy(bv_bcast[:], pb0[:])
            for fn in prefix_units:
                fn()

            next_av = 0
            for g, (qc, p, kt) in enumerate(seq):
                due = av_due(g)
                # av first (its exp finished long before this group's scores
                # dep) -- EXCEPT a chunk's first av, which waits on the
                # previous chunk's norm to free the po region and would clog
                # the PE wait-queue ahead of the scores
                if next_av <= due and seq[next_av][2] != 0:
                    emit_av(next_av, *seq[next_av])
                    next_av += 1
                emit_scores(g, qc, p, kt)
                for fn in sched.get(g, ()):
                    fn()
                while next_av <= due:
                    emit_av(next_av, *seq[next_av])
                    next_av += 1
            i = 0
            while next_av < NG:
                emit_av(next_av, *seq[next_av])
                next_av += 1
                for fn in post_loop.get(i, ()):
                    fn()
                i += 1

    nc.compile()
    return nc


_CACHE = {}


def _get_nc():
    if "nc" not in _CACHE:
        _CACHE["nc"] = build()
    return _CACHE["nc"]


def make_in_maps(x, w_qkv, b_qkv, w_proj, b_proj):
    x = np.asarray(x, dtype=np.float32)
    w_qkv = np.asarray(w_qkv, dtype=np.float32)
    b_qkv = np.asarray(b_qkv, dtype=np.float32)
    w_proj = np.asarray(w_proj, dtype=np.float32)
    b_proj = np.asarray(b_proj, dtype=np.float32)
    in_maps = []
    for c in range(N_CORES):
        b, g = c // 4, c % 4
        f = slice(g * FQ, (g + 1) * FQ)
        fq = slice(g * FQ, (g + 1) * FQ)
        fk = slice(D + g * FQ, D + (g + 1) * FQ)
        fv = slice(2 * D + g * FQ, 2 * D + (g + 1) * FQ)
        in_maps.append(
            {
                "xt": np.ascontiguousarray(x[b].T),
                # column order [Q-pair0 | K-pair0 | Q-pair1 | K-pair1]: the
                # prefix consumes only the first 256 columns, so its DMA
                # piece is contiguous (1KB descriptors) and 1MB smaller
                "wqk": np.ascontiguousarray(
                    np.concatenate(
                        [
                            w_qkv[:, fq][:, :128],
                            w_qkv[:, fk][:, :128],
                            w_qkv[:, fq][:, 128:],
                            w_qkv[:, fk][:, 128:],
                        ],
                        axis=1,
                    )
                ),
                "wv": np.ascontiguousarray(w_qkv[:, fv]),
                "bqk": np.concatenate([b_qkv[fq], b_qkv[fk]]).reshape(2 * FQ, 1).copy(),
                "bv": b_qkv[fv].reshape(1, FQ).copy(),
                "wpr": np.ascontiguousarray(w_proj[f, :]),
                "bpr": (b_proj / 4.0).reshape(1, D).copy(),
            }
        )
    return in_maps


def assemble(results):
    out = np.empty((B, S, D), dtype=np.float32)
    for b in range(B):
        grp = [
            np.asarray(results[4 * b + i]["out"], dtype=np.float32)
            for i in range(4)
        ]
        out[b] = grp[0] + grp[1] + grp[2] + grp[3]
    return out


def kernel(x, w_qkv, b_qkv, w_proj, b_proj, num_heads=H, **_):
    in_maps = make_in_maps(x, w_qkv, b_qkv, w_proj, b_proj)
    res = run_bass_kernel_spmd(
        _get_nc(), in_maps, core_ids=list(range(N_CORES))
    )
    return assemble(res.results)
